# revision 20
# baseline (speedup 1.0000x reference)
"""MaxUnpooling2D scatter-add kernel for Trainium2 (8 NeuronCores).

Reference semantics (per batch b):
    y = mask // (OW*C); x = (mask // C) % OW; f = channel index c
    out[b, y, x, c] += updates[b, h, w, c]      (duplicates sum)

Strategy (pure data-parallel over batch; 2 batches per core):
  - Layout SBUF tiles [128 partitions, 4096] where partition p holds hw rows
    [32p, 32p+32) and free column j = q*128 + c  (q in [0,32), c = channel).
  - For each (plane c, chunk q): the 128 elements (one per partition) are
    scatter-routed with a dense one-hot matmul:
        A[i, y]  = (iota == Y[i])             (stationary operand)
        Bv[i, x] = (iota == X[i]) * V[i]      (moving operand)
        psum_c[y, x] += A.T @ Bv              (PE contraction over i)
    PSUM (f32) accumulates the 32 chunks of a plane; duplicates sum exactly.
  - Evacuate psum_c[y, x] into PL[y, x, c]; one contiguous 8MB DMA per batch.

Engine split: A-builds alternate DVE/GPSIMD, B-builds on DVE, evac on ACT,
one-hots in bf16 (exact for integer lane ids) so the matmul runs the fast
bf16 path; values are bf16-rounded (set VALUE_DTYPE to f32 for exact values
at ~2x the PE cost).
"""

import sys

sys.path.insert(0, "/opt/trn_rl_repo")

import numpy as np

import concourse.bacc as bacc
import concourse.bass as bass
import concourse.tile as tile
from concourse import mybir
from concourse.bass_utils import run_bass_kernel_spmd

# Problem shape (hardcoded per contract)
B, H, W, C = 16, 64, 64, 128
OH, OW = 2 * H, 2 * W
N_CORES = 8
B_PER_CORE = B // N_CORES  # 2
HWF = H * W  # 4096
P = 128
Q = HWF // P  # 32 hw rows per partition
NCOL = Q * C  # 4096

F32 = mybir.dt.float32
BF16 = mybir.dt.bfloat16
FP16 = mybir.dt.float16
I32 = mybir.dt.int32

def build_nc(n_planes=C, repeat=1, dt=FP16):
    nc = bacc.Bacc("TRN2", target_bir_lowering=False, debug=False)

    upd = nc.declare_dram_parameter("updates", [B_PER_CORE, HWF, C], F32, isOutput=False)
    msk = nc.declare_dram_parameter("mask", [B_PER_CORE, HWF, C], I32, isOutput=False)
    iota_in = nc.declare_dram_parameter("iota", [P, P], F32, isOutput=False)
    out = nc.declare_dram_parameter("out", [B_PER_CORE, OH, OW, C], F32, isOutput=True)

    with tile.TileContext(nc) as tc:
        with (
            tc.tile_pool(name="const", bufs=1) as const_pool,
            tc.tile_pool(name="inp", bufs=1) as inp_pool,
            tc.tile_pool(name="pl", bufs=1) as pl_pool,
            tc.tile_pool(name="apool", bufs=3) as a_pool,
            tc.tile_pool(name="bpool", bufs=16) as b_pool,
            tc.tile_pool(name="psum", bufs=8, space="PSUM") as psum_pool,
        ):
            iota_f = const_pool.tile([P, P], F32)
            nc.sync.dma_start(iota_f[:], iota_in[:])
            iota = const_pool.tile([P, P], dt)
            nc.vector.tensor_copy(iota[:], iota_f[:])

            for b_rep in range(B_PER_CORE * repeat):
                b = b_rep % B_PER_CORE
                # ---- load batch b ----
                u_f = inp_pool.tile([P, NCOL], F32, tag="uf")
                nc.sync.dma_start(u_f[:], upd[b].rearrange("(p q) c -> p (q c)", p=P))
                m = inp_pool.tile([P, NCOL], I32, tag="m")
                nc.sync.dma_start(m[:], msk[b].rearrange("(p q) c -> p (q c)", p=P))

                # ---- decode mask -> Y, X in build dtype; values likewise ----
                yi = inp_pool.tile([P, NCOL], I32, tag="yi")
                nc.vector.tensor_scalar(
                    yi[:], m[:], 14, None, mybir.AluOpType.logical_shift_right
                )
                yf = inp_pool.tile([P, NCOL], F32, tag="yf")
                nc.vector.tensor_copy(yf[:], yi[:])

                xi = inp_pool.tile([P, NCOL], I32, tag="xi")
                nc.vector.tensor_scalar(
                    xi[:],
                    m[:],
                    7,
                    127,
                    mybir.AluOpType.logical_shift_right,
                    mybir.AluOpType.bitwise_and,
                )
                xf = inp_pool.tile([P, NCOL], F32, tag="xf")
                nc.vector.tensor_copy(xf[:], xi[:])

                u = u_f  # scalar operands must be f32

                pl = pl_pool.tile([P, P, C], F32)  # [y, x, c]
                if n_planes < C:
                    nc.gpsimd.memset(pl[:], 0.0)

                iota_rep = (
                    iota_f[:]
                    .rearrange("p (o x) -> p o x", o=1)
                    .broadcast_to([P, Q, P])
                )
                yf3 = yf[:].rearrange("p (q c) -> p q c", c=C)

                for c in range(n_planes):
                    # batched A one-hot for the whole plane: one DVE op
                    # A[p, q, y] = (iota[y] == Y[p, q*C+c])
                    a_pl = a_pool.tile([P, Q, P], dt, tag="a")
                    y_bc = yf3[:, :, c].broadcast_to([P, Q, P])
                    nc.vector.tensor_tensor(
                        a_pl[:], iota_rep, y_bc, mybir.AluOpType.is_equal
                    )

                    acc = psum_pool.tile([P, P], F32)  # [y, x]
                    for q in range(Q):
                        j = q * C + c
                        b_t = b_pool.tile([P, P], dt, tag="b")
                        nc.vector.tensor_scalar(
                            b_t[:],
                            iota[:],
                            xf[:, j : j + 1],
                            u[:, j : j + 1],
                            mybir.AluOpType.is_equal,
                            mybir.AluOpType.mult,
                        )
                        # psum[y, x] += sum_i a[i, y] * b_t[i, x]
                        nc.tensor.matmul(
                            acc[:],
                            a_pl[:, q, :],
                            b_t[:],
                            start=(q == 0),
                            stop=(q == Q - 1),
                        )
                    # evacuate plane: pl[:, :, c] = acc
                    nc.scalar.copy(pl[:, :, c], acc[:])

                nc.sync.dma_start(out[b].rearrange("y x c -> y (x c)"), pl[:])

    nc.compile()
    return nc


_CACHED = {}


def _get_nc(n_planes=C):
    key = n_planes
    if key not in _CACHED:
        _CACHED[key] = build_nc(n_planes)
    return _CACHED[key]


def kernel(updates: np.ndarray, mask: np.ndarray) -> np.ndarray:
    nc = _get_nc()
    iota = np.broadcast_to(np.arange(P, dtype=np.float32), (P, P)).copy()
    in_maps = []
    for i in range(N_CORES):
        sl = slice(i * B_PER_CORE, (i + 1) * B_PER_CORE)
        in_maps.append(
            {
                "updates": np.ascontiguousarray(
                    updates[sl].reshape(B_PER_CORE, HWF, C), dtype=np.float32
                ),
                "mask": np.ascontiguousarray(
                    mask[sl].reshape(B_PER_CORE, HWF, C), dtype=np.int32
                ),
                "iota": iota,
            }
        )
    res = run_bass_kernel_spmd(nc, in_maps, list(range(N_CORES)))
    return np.concatenate([res.results[i]["out"] for i in range(N_CORES)], axis=0)


# revision 21
# speedup vs baseline: 1.0012x; 1.0012x over previous
"""MaxUnpooling2D scatter-add kernel for Trainium2 (8 NeuronCores).

Reference semantics (per batch b):
    y = mask // (OW*C); x = (mask // C) % OW; f = channel index c
    out[b, y, x, c] += updates[b, h, w, c]      (duplicates sum)

Strategy (pure data-parallel over batch; 2 batches per core):
  - Layout SBUF tiles [128 partitions, 4096] where partition p holds hw rows
    [32p, 32p+32) and free column j = q*128 + c  (q in [0,32), c = channel).
  - For each (plane c, chunk q): the 128 elements (one per partition) are
    scatter-routed with a dense one-hot matmul:
        A[i, y]  = (iota == Y[i])             (stationary operand)
        Bv[i, x] = (iota == X[i]) * V[i]      (moving operand)
        psum_c[y, x] += A.T @ Bv              (PE contraction over i)
    PSUM (f32) accumulates the 32 chunks of a plane; duplicates sum exactly.
  - Evacuate psum_c[y, x] into PL[y, x, c]; one contiguous 8MB DMA per batch.

Engine split: the A one-hot is built for a whole plane in ONE batched DVE
tensor_tensor (broadcast access patterns), B per chunk as a fused DVE
tensor_scalar (is_equal, mult), PSUM evac on ACT. One-hots and values are
fp16 (lane ids exact; values rounded to 11 bits -> ~2e-4 rel err) so the
matmul runs the fast 16-bit PE path; PSUM accumulates in f32. Measured
~3.0 ms HW exec (DVE-bound at ~99% occupancy); dt=F32 gives exact values
at ~3.8 ms (fp32 matmul runs as two PE passes).
"""

import sys

sys.path.insert(0, "/opt/trn_rl_repo")

import numpy as np

import concourse.bacc as bacc
import concourse.bass as bass
import concourse.tile as tile
from concourse import mybir
from concourse.bass_utils import run_bass_kernel_spmd

# Problem shape (hardcoded per contract)
B, H, W, C = 16, 64, 64, 128
OH, OW = 2 * H, 2 * W
N_CORES = 8
B_PER_CORE = B // N_CORES  # 2
HWF = H * W  # 4096
P = 128
Q = HWF // P  # 32 hw rows per partition
NCOL = Q * C  # 4096

F32 = mybir.dt.float32
BF16 = mybir.dt.bfloat16
FP16 = mybir.dt.float16
I32 = mybir.dt.int32

def build_nc(n_planes=C, repeat=1, dt=FP16):
    nc = bacc.Bacc("TRN2", target_bir_lowering=False, debug=False)

    upd = nc.declare_dram_parameter("updates", [B_PER_CORE, HWF, C], F32, isOutput=False)
    msk = nc.declare_dram_parameter("mask", [B_PER_CORE, HWF, C], I32, isOutput=False)
    iota_in = nc.declare_dram_parameter("iota", [P, P], F32, isOutput=False)
    out = nc.declare_dram_parameter("out", [B_PER_CORE, OH, OW, C], F32, isOutput=True)

    with tile.TileContext(nc) as tc:
        with (
            tc.tile_pool(name="const", bufs=1) as const_pool,
            tc.tile_pool(name="inp", bufs=1) as inp_pool,
            tc.tile_pool(name="pl", bufs=1) as pl_pool,
            tc.tile_pool(name="apool", bufs=3) as a_pool,
            tc.tile_pool(name="bpool", bufs=16) as b_pool,
            tc.tile_pool(name="psum", bufs=8, space="PSUM") as psum_pool,
        ):
            iota_f = const_pool.tile([P, P], F32)
            nc.sync.dma_start(iota_f[:], iota_in[:])
            iota = const_pool.tile([P, P], dt)
            nc.vector.tensor_copy(iota[:], iota_f[:])

            for b_rep in range(B_PER_CORE * repeat):
                b = b_rep % B_PER_CORE
                # ---- load batch b ----
                u_f = inp_pool.tile([P, NCOL], F32, tag="uf")
                nc.sync.dma_start(u_f[:], upd[b].rearrange("(p q) c -> p (q c)", p=P))
                m = inp_pool.tile([P, NCOL], I32, tag="m")
                nc.sync.dma_start(m[:], msk[b].rearrange("(p q) c -> p (q c)", p=P))

                # ---- decode mask -> Y, X in build dtype; values likewise ----
                yi = inp_pool.tile([P, NCOL], I32, tag="yi")
                nc.vector.tensor_scalar(
                    yi[:], m[:], 14, None, mybir.AluOpType.logical_shift_right
                )
                yf = inp_pool.tile([P, NCOL], F32, tag="yf")
                nc.vector.tensor_copy(yf[:], yi[:])

                xi = inp_pool.tile([P, NCOL], I32, tag="xi")
                nc.vector.tensor_scalar(
                    xi[:],
                    m[:],
                    7,
                    127,
                    mybir.AluOpType.logical_shift_right,
                    mybir.AluOpType.bitwise_and,
                )
                xf = inp_pool.tile([P, NCOL], F32, tag="xf")
                nc.vector.tensor_copy(xf[:], xi[:])

                u = u_f  # scalar operands must be f32

                pl = pl_pool.tile([P, P, C], F32)  # [y, x, c]
                if n_planes < C:
                    nc.gpsimd.memset(pl[:], 0.0)

                iota_rep = (
                    iota_f[:]
                    .rearrange("p (o x) -> p o x", o=1)
                    .broadcast_to([P, Q, P])
                )
                yf3 = yf[:].rearrange("p (q c) -> p q c", c=C)

                for c in range(n_planes):
                    # batched A one-hot for the whole plane: one DVE op
                    # A[p, q, y] = (iota[y] == Y[p, q*C+c])
                    a_pl = a_pool.tile([P, Q, P], dt, tag="a")
                    y_bc = yf3[:, :, c].broadcast_to([P, Q, P])
                    nc.vector.tensor_tensor(
                        a_pl[:], iota_rep, y_bc, mybir.AluOpType.is_equal
                    )

                    acc = psum_pool.tile([P, P], F32)  # [y, x]
                    for q in range(Q):
                        j = q * C + c
                        b_t = b_pool.tile([P, P], dt, tag="b")
                        nc.vector.tensor_scalar(
                            b_t[:],
                            iota[:],
                            xf[:, j : j + 1],
                            u[:, j : j + 1],
                            mybir.AluOpType.is_equal,
                            mybir.AluOpType.mult,
                        )
                        # psum[y, x] += sum_i a[i, y] * b_t[i, x]
                        nc.tensor.matmul(
                            acc[:],
                            a_pl[:, q, :],
                            b_t[:],
                            start=(q == 0),
                            stop=(q == Q - 1),
                        )
                    # evacuate plane: pl[:, :, c] = acc
                    nc.scalar.copy(pl[:, :, c], acc[:])

                nc.sync.dma_start(out[b].rearrange("y x c -> y (x c)"), pl[:])

    nc.compile()
    return nc


_CACHED = {}


def _get_nc(n_planes=C):
    key = n_planes
    if key not in _CACHED:
        _CACHED[key] = build_nc(n_planes)
    return _CACHED[key]


def kernel(updates: np.ndarray, mask: np.ndarray) -> np.ndarray:
    nc = _get_nc()
    iota = np.broadcast_to(np.arange(P, dtype=np.float32), (P, P)).copy()
    in_maps = []
    for i in range(N_CORES):
        sl = slice(i * B_PER_CORE, (i + 1) * B_PER_CORE)
        in_maps.append(
            {
                "updates": np.ascontiguousarray(
                    updates[sl].reshape(B_PER_CORE, HWF, C), dtype=np.float32
                ),
                "mask": np.ascontiguousarray(
                    mask[sl].reshape(B_PER_CORE, HWF, C), dtype=np.int32
                ),
                "iota": iota,
            }
        )
    res = run_bass_kernel_spmd(nc, in_maps, list(range(N_CORES)))
    return np.concatenate([res.results[i]["out"] for i in range(N_CORES)], axis=0)


# revision 27
# speedup vs baseline: 1.5232x; 1.5214x over previous
"""MaxUnpooling2D scatter-add kernel for Trainium2 (8 NeuronCores).

Reference semantics (per batch b):
    y = mask // (OW*C); x = (mask // C) % OW; f = channel index c
    out[b, y, x, c] += updates[b, h, w, c]      (duplicates sum)

Strategy (pure data-parallel over batch; 2 batches per core):
  - Layout SBUF tiles [128 partitions, 4096] where partition p holds hw rows
    [32p, 32p+32) and free column j = q*128 + c  (q in [0,32), c = channel).
  - For each (plane c, chunk q): the 128 elements (one per partition) are
    scatter-routed with a dense one-hot matmul:
        A[i, y]  = (iota == Y[i])             (stationary operand)
        Bv[i, x] = (iota == X[i]) * V[i]      (moving operand)
        psum_c[y, x] += A.T @ Bv              (PE contraction over i)
    PSUM (f32) accumulates the 32 chunks of a plane; duplicates sum exactly.
  - Evacuate psum_c[y, x] into PL[y, x, c]; one contiguous 8MB DMA per batch.

Engine split: the A one-hot is built for a whole plane in ONE batched DVE
tensor_tensor (broadcast access patterns), B per chunk as a fused DVE
tensor_scalar (is_equal, mult), PSUM evac on ACT. One-hots and values are
fp16 (lane ids exact; values rounded to 11 bits -> ~2e-4 rel err) so the
matmul runs the fast 16-bit PE path; PSUM accumulates in f32. Measured
~3.0 ms HW exec (DVE-bound at ~99% occupancy); dt=F32 gives exact values
at ~3.8 ms (fp32 matmul runs as two PE passes).
"""

import sys

sys.path.insert(0, "/opt/trn_rl_repo")

import numpy as np

import concourse.bacc as bacc
import concourse.bass as bass
import concourse.tile as tile
from concourse import mybir
from concourse.bass_utils import run_bass_kernel_spmd

# Problem shape (hardcoded per contract)
B, H, W, C = 16, 64, 64, 128
OH, OW = 2 * H, 2 * W
N_CORES = 8
B_PER_CORE = B // N_CORES  # 2
HWF = H * W  # 4096
P = 128
Q = HWF // P  # 32 hw rows per partition
NCOL = Q * C  # 4096

F32 = mybir.dt.float32
BF16 = mybir.dt.bfloat16
FP16 = mybir.dt.float16
I32 = mybir.dt.int32

def build_nc(n_planes=C, repeat=1, dt=FP16):
    nc = bacc.Bacc("TRN2", target_bir_lowering=False, debug=False)

    upd = nc.declare_dram_parameter("updates", [B_PER_CORE, HWF, C], F32, isOutput=False)
    msk = nc.declare_dram_parameter("mask", [B_PER_CORE, HWF, C], I32, isOutput=False)
    iota_in = nc.declare_dram_parameter("iota", [P, P], F32, isOutput=False)
    out = nc.declare_dram_parameter("out", [B_PER_CORE, OH, OW, C], F32, isOutput=True)

    with tile.TileContext(nc) as tc:
        with (
            tc.tile_pool(name="const", bufs=1) as const_pool,
            tc.tile_pool(name="inp", bufs=1) as inp_pool,
            tc.tile_pool(name="pl", bufs=1) as pl_pool,
            tc.tile_pool(name="apool", bufs=3) as a_pool,
            tc.tile_pool(name="xpool", bufs=2) as x_pool,
            tc.tile_pool(name="bpool", bufs=2) as b_pool,
            tc.tile_pool(name="psum", bufs=8, space="PSUM") as psum_pool,
        ):
            iota_f = const_pool.tile([P, P], F32)
            nc.sync.dma_start(iota_f[:], iota_in[:])
            # materialized iotaT[p, y, q] = y  (fp16, innermost step 1) so the
            # plane-batched build ops qualify for the DVE 2x packed mode
            iotaT = const_pool.tile([P, P, Q], dt)
            nc.vector.tensor_copy(
                iotaT[:],
                iota_f[:]
                .rearrange("p (y o) -> p y o", o=1)
                .broadcast_to([P, P, Q]),
            )

            for b_rep in range(B_PER_CORE * repeat):
                b = b_rep % B_PER_CORE
                # ---- load batch b ----
                u_f = inp_pool.tile([P, NCOL], F32, tag="uf")
                nc.sync.dma_start(u_f[:], upd[b].rearrange("(p q) c -> p (q c)", p=P))
                m = inp_pool.tile([P, NCOL], I32, tag="m")
                nc.sync.dma_start(m[:], msk[b].rearrange("(p q) c -> p (q c)", p=P))

                # ---- decode mask -> channel-major fp16 Y/X/V tiles [p, c, q] ----
                yi = inp_pool.tile([P, NCOL], I32, tag="yi")
                nc.vector.tensor_scalar(
                    yi[:], m[:], 14, None, mybir.AluOpType.logical_shift_right
                )
                ytr = inp_pool.tile([P, C, Q], dt, tag="ytr")
                nc.vector.tensor_copy(ytr[:], yi[:].rearrange("p (q c) -> p c q", c=C))

                xi = inp_pool.tile([P, NCOL], I32, tag="yi")
                nc.vector.tensor_scalar(
                    xi[:],
                    m[:],
                    7,
                    127,
                    mybir.AluOpType.logical_shift_right,
                    mybir.AluOpType.bitwise_and,
                )
                xtr = inp_pool.tile([P, C, Q], dt, tag="xtr")
                nc.vector.tensor_copy(xtr[:], xi[:].rearrange("p (q c) -> p c q", c=C))

                vtr = inp_pool.tile([P, C, Q], dt, tag="vtr")
                nc.vector.tensor_copy(vtr[:], u_f[:].rearrange("p (q c) -> p c q", c=C))

                pl = pl_pool.tile([P, P, C], F32)  # [y, x, c]
                if n_planes < C:
                    nc.gpsimd.memset(pl[:], 0.0)

                for c in range(n_planes):
                    # plane-batched builds, all at DVE 2x (fp16, step-1 inner):
                    # a[p, y, q] = (iotaT == Ytr[p,c,q]); b = (iotaT == X) * V
                    a_pl = a_pool.tile([P, P, Q], dt, tag="a")
                    y_bc = (
                        ytr[:, c, :]
                        .rearrange("p (o q) -> p o q", o=1)
                        .broadcast_to([P, P, Q])
                    )
                    nc.vector.tensor_tensor(
                        a_pl[:], iotaT[:], y_bc, mybir.AluOpType.is_equal
                    )
                    xeq = x_pool.tile([P, P, Q], dt, tag="xeq")
                    x_bc = (
                        xtr[:, c, :]
                        .rearrange("p (o q) -> p o q", o=1)
                        .broadcast_to([P, P, Q])
                    )
                    nc.vector.tensor_tensor(
                        xeq[:], iotaT[:], x_bc, mybir.AluOpType.is_equal
                    )
                    b_pl = b_pool.tile([P, P, Q], dt, tag="b")
                    v_bc = (
                        vtr[:, c, :]
                        .rearrange("p (o q) -> p o q", o=1)
                        .broadcast_to([P, P, Q])
                    )
                    nc.vector.tensor_tensor(
                        b_pl[:], xeq[:], v_bc, mybir.AluOpType.mult
                    )

                    acc = psum_pool.tile([P, P], F32)  # [y, x]
                    for q in range(Q):
                        # psum[y, x] += sum_i a[i, y] * b[i, x]
                        nc.tensor.matmul(
                            acc[:],
                            a_pl[:, :, q],
                            b_pl[:, :, q],
                            start=(q == 0),
                            stop=(q == Q - 1),
                        )
                    # evacuate plane: pl[:, :, c] = acc
                    nc.scalar.copy(pl[:, :, c], acc[:])

                nc.sync.dma_start(out[b].rearrange("y x c -> y (x c)"), pl[:])

    nc.compile()
    return nc


_CACHED = {}


def _get_nc(n_planes=C):
    key = n_planes
    if key not in _CACHED:
        _CACHED[key] = build_nc(n_planes)
    return _CACHED[key]


def kernel(updates: np.ndarray, mask: np.ndarray) -> np.ndarray:
    nc = _get_nc()
    iota = np.broadcast_to(np.arange(P, dtype=np.float32), (P, P)).copy()
    in_maps = []
    for i in range(N_CORES):
        sl = slice(i * B_PER_CORE, (i + 1) * B_PER_CORE)
        in_maps.append(
            {
                "updates": np.ascontiguousarray(
                    updates[sl].reshape(B_PER_CORE, HWF, C), dtype=np.float32
                ),
                "mask": np.ascontiguousarray(
                    mask[sl].reshape(B_PER_CORE, HWF, C), dtype=np.int32
                ),
                "iota": iota,
            }
        )
    res = run_bass_kernel_spmd(nc, in_maps, list(range(N_CORES)))
    return np.concatenate([res.results[i]["out"] for i in range(N_CORES)], axis=0)


# revision 31
# speedup vs baseline: 1.5259x; 1.0018x over previous
"""MaxUnpooling2D scatter-add kernel for Trainium2 (8 NeuronCores).

Reference semantics (per batch b):
    y = mask // (OW*C); x = (mask // C) % OW; f = channel index c
    out[b, y, x, c] += updates[b, h, w, c]      (duplicates sum)

Strategy (pure data-parallel over batch; 2 batches per core):
  - Layout SBUF tiles [128 partitions, 4096] where partition p holds hw rows
    [32p, 32p+32) and free column j = q*128 + c  (q in [0,32), c = channel).
  - For each (plane c, chunk q): the 128 elements (one per partition) are
    scatter-routed with a dense one-hot matmul:
        A[i, y]  = (iota == Y[i])             (stationary operand)
        Bv[i, x] = (iota == X[i]) * V[i]      (moving operand)
        psum_c[y, x] += A.T @ Bv              (PE contraction over i)
    PSUM (f32) accumulates the 32 chunks of a plane; duplicates sum exactly.
  - Evacuate psum_c[y, x] into PL[y, x, c]; one contiguous 8MB DMA per batch.

Engine split: all one-hot builds are plane-batched DVE tensor_tensor ops in
a transposed [partition, onehot, q] layout with a MATERIALIZED iota tensor,
so every operand has an innermost step of 1 in fp16 and the ops run in the
DVE 2x packed mode (broadcasts ride on middle dims only). The matmul takes
strided [p, :, q] slices (strided LDWEIGHTS costs ~40 ns, acceptable).
PSUM evac on ACT. fp16 one-hots/values (lane ids exact; values rounded to
11 bits -> ~2e-4 rel err); PSUM accumulates in f32. Measured 1.97 ms HW
exec (DVE ~92% busy at the batched-2x floor); pool depths (apool=3,
xpool=2, bpool=2, psum=8) are load-bearing for PE/DVE overlap.
"""

import sys

sys.path.insert(0, "/opt/trn_rl_repo")

import numpy as np

import concourse.bacc as bacc
import concourse.bass as bass
import concourse.tile as tile
from concourse import mybir
from concourse.bass_utils import run_bass_kernel_spmd

# Problem shape (hardcoded per contract)
B, H, W, C = 16, 64, 64, 128
OH, OW = 2 * H, 2 * W
N_CORES = 8
B_PER_CORE = B // N_CORES  # 2
HWF = H * W  # 4096
P = 128
Q = HWF // P  # 32 hw rows per partition
NCOL = Q * C  # 4096

F32 = mybir.dt.float32
BF16 = mybir.dt.bfloat16
FP16 = mybir.dt.float16
I32 = mybir.dt.int32

def build_nc(n_planes=C, repeat=1, dt=FP16):
    nc = bacc.Bacc("TRN2", target_bir_lowering=False, debug=False)

    upd = nc.declare_dram_parameter("updates", [B_PER_CORE, HWF, C], F32, isOutput=False)
    msk = nc.declare_dram_parameter("mask", [B_PER_CORE, HWF, C], I32, isOutput=False)
    iota_in = nc.declare_dram_parameter("iota", [P, P], F32, isOutput=False)
    out = nc.declare_dram_parameter("out", [B_PER_CORE, OH, OW, C], F32, isOutput=True)

    with tile.TileContext(nc) as tc:
        with (
            tc.tile_pool(name="const", bufs=1) as const_pool,
            tc.tile_pool(name="inp", bufs=1) as inp_pool,
            tc.tile_pool(name="pl", bufs=1) as pl_pool,
            tc.tile_pool(name="apool", bufs=3) as a_pool,
            tc.tile_pool(name="xpool", bufs=2) as x_pool,
            tc.tile_pool(name="bpool", bufs=2) as b_pool,
            tc.tile_pool(name="psum", bufs=8, space="PSUM") as psum_pool,
        ):
            iota_f = const_pool.tile([P, P], F32)
            nc.sync.dma_start(iota_f[:], iota_in[:])
            # materialized iotaT[p, y, q] = y  (fp16, innermost step 1) so the
            # plane-batched build ops qualify for the DVE 2x packed mode
            iotaT = const_pool.tile([P, P, Q], dt)
            nc.vector.tensor_copy(
                iotaT[:],
                iota_f[:]
                .rearrange("p (y o) -> p y o", o=1)
                .broadcast_to([P, P, Q]),
            )

            for b_rep in range(B_PER_CORE * repeat):
                b = b_rep % B_PER_CORE
                # ---- load batch b ----
                u_f = inp_pool.tile([P, NCOL], F32, tag="uf")
                nc.sync.dma_start(u_f[:], upd[b].rearrange("(p q) c -> p (q c)", p=P))
                m = inp_pool.tile([P, NCOL], I32, tag="m")
                nc.sync.dma_start(m[:], msk[b].rearrange("(p q) c -> p (q c)", p=P))

                # ---- decode mask -> channel-major fp16 Y/X/V tiles [p, c, q] ----
                yi = inp_pool.tile([P, NCOL], I32, tag="yi")
                nc.vector.tensor_scalar(
                    yi[:], m[:], 14, None, mybir.AluOpType.logical_shift_right
                )
                ytr = inp_pool.tile([P, C, Q], dt, tag="ytr")
                nc.vector.tensor_copy(ytr[:], yi[:].rearrange("p (q c) -> p c q", c=C))

                xi = inp_pool.tile([P, NCOL], I32, tag="yi")
                nc.vector.tensor_scalar(
                    xi[:],
                    m[:],
                    7,
                    127,
                    mybir.AluOpType.logical_shift_right,
                    mybir.AluOpType.bitwise_and,
                )
                xtr = inp_pool.tile([P, C, Q], dt, tag="xtr")
                nc.vector.tensor_copy(xtr[:], xi[:].rearrange("p (q c) -> p c q", c=C))

                vtr = inp_pool.tile([P, C, Q], dt, tag="vtr")
                nc.vector.tensor_copy(vtr[:], u_f[:].rearrange("p (q c) -> p c q", c=C))

                pl = pl_pool.tile([P, P, C], F32)  # [y, x, c]
                if n_planes < C:
                    nc.gpsimd.memset(pl[:], 0.0)

                for c in range(n_planes):
                    # plane-batched builds, all at DVE 2x (fp16, step-1 inner):
                    # a[p, y, q] = (iotaT == Ytr[p,c,q]); b = (iotaT == X) * V
                    a_pl = a_pool.tile([P, P, Q], dt, tag="a")
                    y_bc = (
                        ytr[:, c, :]
                        .rearrange("p (o q) -> p o q", o=1)
                        .broadcast_to([P, P, Q])
                    )
                    nc.vector.tensor_tensor(
                        a_pl[:], iotaT[:], y_bc, mybir.AluOpType.is_equal
                    )
                    xeq = x_pool.tile([P, P, Q], dt, tag="xeq")
                    x_bc = (
                        xtr[:, c, :]
                        .rearrange("p (o q) -> p o q", o=1)
                        .broadcast_to([P, P, Q])
                    )
                    nc.vector.tensor_tensor(
                        xeq[:], iotaT[:], x_bc, mybir.AluOpType.is_equal
                    )
                    b_pl = b_pool.tile([P, P, Q], dt, tag="b")
                    v_bc = (
                        vtr[:, c, :]
                        .rearrange("p (o q) -> p o q", o=1)
                        .broadcast_to([P, P, Q])
                    )
                    nc.vector.tensor_tensor(
                        b_pl[:], xeq[:], v_bc, mybir.AluOpType.mult
                    )

                    acc = psum_pool.tile([P, P], F32)  # [y, x]
                    for q in range(Q):
                        # psum[y, x] += sum_i a[i, y] * b[i, x]
                        nc.tensor.matmul(
                            acc[:],
                            a_pl[:, :, q],
                            b_pl[:, :, q],
                            start=(q == 0),
                            stop=(q == Q - 1),
                        )
                    # evacuate plane: pl[:, :, c] = acc
                    nc.scalar.copy(pl[:, :, c], acc[:])

                nc.sync.dma_start(out[b].rearrange("y x c -> y (x c)"), pl[:])

    nc.compile()
    return nc


_CACHED = {}


def _get_nc(n_planes=C):
    key = n_planes
    if key not in _CACHED:
        _CACHED[key] = build_nc(n_planes)
    return _CACHED[key]


def kernel(updates: np.ndarray, mask: np.ndarray) -> np.ndarray:
    nc = _get_nc()
    iota = np.broadcast_to(np.arange(P, dtype=np.float32), (P, P)).copy()
    in_maps = []
    for i in range(N_CORES):
        sl = slice(i * B_PER_CORE, (i + 1) * B_PER_CORE)
        in_maps.append(
            {
                "updates": np.ascontiguousarray(
                    updates[sl].reshape(B_PER_CORE, HWF, C), dtype=np.float32
                ),
                "mask": np.ascontiguousarray(
                    mask[sl].reshape(B_PER_CORE, HWF, C), dtype=np.int32
                ),
                "iota": iota,
            }
        )
    res = run_bass_kernel_spmd(nc, in_maps, list(range(N_CORES)))
    return np.concatenate([res.results[i]["out"] for i in range(N_CORES)], axis=0)
